# revision 24
# baseline (speedup 1.0000x reference)
"""2-layer GCN (GCNConv 128->128->64, N=50000, E=800000) on 8 TRN2 NeuronCores.

v4 strategy (dst-sharded, aggregate-first, gather-direct):
  out = relu(A_hat @ (relu(A_hat @ x @ W1 + b1)) @ W2 + b2),  A_hat = D^-1/2 (A+I) D^-1/2
  - Layer 1 gathers source features DIRECTLY from the replicated f32 input x
    (512B rows, no table build, no AllGather #1). ScalarE converts f32->bf16.
  - Per-edge norm folded into the one-hot scatter matrices (VectorE, 2x mode:
    one-hot laid out [edge, dst, tile] so every operand has a packed last dim,
    iota materialized as a const [P, P, SMAX] tile).
  - Layer 2: x2' = Dis*relu(h1) computed dst-sharded, exchanged via TWO chunked
    AllGathers (src-block ranges 0:25 / 25:49); AllGather #0 is emitted into the
    Pool stream mid-layer-1 so it overlaps the layer-1 tail; layer 2 runs two
    passes (one per chunk table) accumulating into a bf16 SBUF accumulator.
  - Edges sorted by dst, sharded across 8 cores by dst range (6250/core), dst
    blocks of 128; scatter-add via PSUM-accumulated TensorE matmuls.
  - Gathers in 8-tile (1024-idx) dma_gather calls (HW packet limit: 64
    descriptors/engine), 4 SWDGE queues round-robin.
Host-side work is index-only prep (sort/degree-histogram/plan) + output concat.
"""

import numpy as np
import ml_dtypes

import concourse.bass as bass
import concourse.bacc as bacc
import concourse.mybir as mybir
import concourse.tile as tile
from concourse.bass_utils import run_bass_kernel_spmd
from concourse.library_config import mlp
from concourse.masks import make_identity

P = 128
N_NODES = 50000
N_EDGES = 800000
IN_CH = 128
HID_CH = 128
OUT_CH = 64
N_CORES = 8
NSH = N_NODES // N_CORES          # 6250 nodes per core
NBLK = (NSH + P - 1) // P         # 49 dst blocks per core (48 full + 106)
NFULL = NSH // P                  # 48
NTAIL = NSH - NFULL * P           # 106
VLO = 32768                       # layer-1 table halves (int16 index range)
GS = 2                            # dst blocks per gather group
NG = (NBLK + GS - 1) // GS        # 25 groups (last has 1 block)
GCAP = 8                          # max tiles (128 idxs each) per dma_gather call
CLEN = [25, 24]                   # layer-2 src-block chunks (blocks 0:25, 25:49)
CSTART = [0, 25]
AG0_AFTER_GROUP = 13              # emit chunk-0 AllGather after this many groups

BF16 = mybir.dt.bfloat16
F32 = mybir.dt.float32

LAST_RESULT = None  # for test harness: BassKernelResults of last run


def _plan(cnt):
    t = np.ceil(cnt.max(axis=0) / P).astype(np.int64)
    off = np.concatenate([[0], np.cumsum(t)])[:-1]
    return t, off, int(t.sum())


def _host_prep(edge_index):
    """Index-only preprocessing. Returns per-core upload arrays + tile plans."""
    src = edge_index[0].astype(np.int64)
    dst = edge_index[1].astype(np.int64)

    deg = np.bincount(dst, minlength=N_NODES) + 1
    dis = (1.0 / np.sqrt(deg.astype(np.float64))).astype(np.float64)

    core = dst // NSH
    i_dst = dst - core * NSH
    blk = i_dst // P
    grp = blk // GS
    big = blk - grp * GS

    # ---- layer 1: table is x itself, halves at VLO ----
    half = (src >= VLO).astype(np.int64)
    row1 = np.where(half == 1, src - VLO, src)
    kk1 = ((core * NG + grp) * 2 + half) * GS + big      # [E]
    NSLOT = NG * 2 * GS                                   # 100 per core
    cnt1 = np.zeros((N_CORES, NSLOT), np.int64)
    np.add.at(cnt1, (core, kk1 - core * NSLOT), 1)
    t1, off1, T1 = _plan(cnt1)

    # ---- layer 2: chunked tables; row = (ksrc*128+psrc)*len_c + (bsrc-cstart) ----
    ksrc = src // NSH
    isrc = src - ksrc * NSH
    bsrc = isrc // P
    psrc = isrc - bsrc * P
    ch = (bsrc >= CLEN[0]).astype(np.int64)
    lenc = np.where(ch == 1, CLEN[1], CLEN[0])
    row2 = (ksrc * P + psrc) * lenc + (bsrc - ch * CSTART[1])
    kk2 = ((core * NG + grp) * 2 + ch) * GS + big
    cnt2 = np.zeros((N_CORES, NSLOT), np.int64)
    np.add.at(cnt2, (core, kk2 - core * NSLOT), 1)
    t2, off2, T2 = _plan(cnt2)

    sc_l1 = (dis[src] * dis[dst]).astype(np.float32)   # table1 = raw x
    sc_l2 = dis[dst].astype(np.float32)                # table2 = Dis*relu(h1)
    drel = (dst - (core * NSH + blk * P)).astype(np.float32)

    def build(kkey, cnt, tiles, offs, T, rows, sc):
        order = np.argsort(kkey, kind="stable")
        ks = kkey[order]
        group_start = np.concatenate([[0], np.cumsum(cnt.reshape(-1))])[:-1]
        pos = np.arange(len(ks)) - group_start[ks]
        slot_in_core = offs[ks % NSLOT] * P + pos
        ecore = ks // NSLOT
        EPC = T * P
        idx_rows = np.zeros((N_CORES, EPC), np.int64)
        dr = np.full((N_CORES, EPC), -1.0, np.float32)
        scl = np.zeros((N_CORES, EPC), np.float32)
        idx_rows[ecore, slot_in_core] = rows[order]
        dr[ecore, slot_in_core] = drel[order]
        scl[ecore, slot_in_core] = sc[order]
        idxw = np.zeros((N_CORES, 16, T * 8), np.int16)
        ii = np.arange(EPC)
        for k in range(N_CORES):
            w = np.zeros((16, T * 8), np.int16)
            w[ii % 16, ii // 16] = idx_rows[k]
            idxw[k] = w
        idxw = np.tile(idxw, (1, 8, 1))
        drw = dr.reshape(N_CORES, T, P).transpose(0, 2, 1)
        scw = scl.reshape(N_CORES, T, P).transpose(0, 2, 1)
        return idxw.astype(np.int16), drw, scw

    idxw1, dr1, sc1 = build(kk1, cnt1, t1, off1, T1, row1, sc_l1)
    idxw2, dr2, sc2 = build(kk2, cnt2, t2, off2, T2, row2, sc_l2)

    disw = np.zeros((N_CORES, P, NBLK), np.float32)
    nodes = np.arange(NBLK * P)
    valid = nodes < NSH
    disf = dis.astype(np.float32)
    for k in range(N_CORES):
        v = np.zeros(NBLK * P, np.float32)
        v[valid] = disf[k * NSH + nodes[valid]]
        disw[k] = v.reshape(NBLK, P).T

    return {
        "t1": t1, "off1": off1, "T1": T1,
        "t2": t2, "off2": off2, "T2": T2,
        "idxw1": idxw1, "dr1": dr1.astype(ml_dtypes.bfloat16),
        "sc1": sc1.astype(ml_dtypes.bfloat16),
        "idxw2": idxw2, "dr2": dr2.astype(ml_dtypes.bfloat16),
        "sc2": sc2.astype(ml_dtypes.bfloat16),
        "disw": disw,
    }


def _make_in_maps(x, W1, b1, W2, b2, prep):
    in_maps = []
    xf = np.ascontiguousarray(x, dtype=np.float32)
    for k in range(N_CORES):
        in_maps.append({
            "x": xf,
            "xsh": np.ascontiguousarray(xf[k * NSH:(k + 1) * NSH]),
            "w1": np.ascontiguousarray(W1, dtype=np.float32),
            "b1": np.ascontiguousarray(np.asarray(b1, np.float32).reshape(HID_CH, 1)),
            "w2": np.ascontiguousarray(W2, dtype=np.float32),
            "b2": np.ascontiguousarray(np.asarray(b2, np.float32).reshape(OUT_CH, 1)),
            "disw": np.ascontiguousarray(prep["disw"][k]),
            "idxw1": np.ascontiguousarray(prep["idxw1"][k]),
            "dr1": np.ascontiguousarray(prep["dr1"][k]),
            "sc1": np.ascontiguousarray(prep["sc1"][k]),
            "idxw2": np.ascontiguousarray(prep["idxw2"][k]),
            "dr2": np.ascontiguousarray(prep["dr2"][k]),
            "sc2": np.ascontiguousarray(prep["sc2"][k]),
        })
    return in_maps


def _build(prep):
    t1, off1, T1 = prep["t1"], prep["off1"], prep["T1"]
    t2, off2, T2 = prep["t2"], prep["off2"], prep["T2"]

    nc = bacc.Bacc("TRN2", target_bir_lowering=False, num_devices=N_CORES,
                   num_swdge_queues=4)

    t_x = nc.dram_tensor("x", [N_NODES, IN_CH], F32, kind="ExternalInput")
    t_xsh = nc.dram_tensor("xsh", [NSH, IN_CH], F32, kind="ExternalInput")
    t_w1 = nc.dram_tensor("w1", [IN_CH, HID_CH], F32, kind="ExternalInput")
    t_b1 = nc.dram_tensor("b1", [HID_CH, 1], F32, kind="ExternalInput")
    t_w2 = nc.dram_tensor("w2", [HID_CH, OUT_CH], F32, kind="ExternalInput")
    t_b2 = nc.dram_tensor("b2", [OUT_CH, 1], F32, kind="ExternalInput")
    t_disw = nc.dram_tensor("disw", [P, NBLK], F32, kind="ExternalInput")
    t_idx1 = nc.dram_tensor("idxw1", [P, T1 * 8], mybir.dt.int16, kind="ExternalInput")
    t_dr1 = nc.dram_tensor("dr1", [P, T1], BF16, kind="ExternalInput")
    t_sc1 = nc.dram_tensor("sc1", [P, T1], BF16, kind="ExternalInput")
    t_idx2 = nc.dram_tensor("idxw2", [P, T2 * 8], mybir.dt.int16, kind="ExternalInput")
    t_dr2 = nc.dram_tensor("dr2", [P, T2], BF16, kind="ExternalInput")
    t_sc2 = nc.dram_tensor("sc2", [P, T2], BF16, kind="ExternalInput")
    t_out = nc.dram_tensor("out", [NSH, OUT_CH], F32, kind="ExternalOutput")

    x2s = [nc.dram_tensor(f"x2s_{c}", [P, CLEN[c] * HID_CH], BF16) for c in range(2)]
    x2f = [nc.dram_tensor(f"x2f_{c}", [N_CORES * P, CLEN[c] * HID_CH], BF16)
           for c in range(2)]

    rg = [list(range(N_CORES))]

    def slot(g, h, big):
        return (g * 2 + h) * GS + big

    def group_meta(tiles, offs):
        meta = []
        for g in range(NG):
            bigs = list(range(min(GS, NBLK - g * GS)))
            Tg = int(offs[slot(g, 0, 0)])
            S = [0, 0]
            rel = {}
            pos = 0
            for h in range(2):
                for big in bigs:
                    n = int(tiles[slot(g, h, big)])
                    rel[(h, big)] = (pos, n)
                    S[h] += n
                    pos += n
            meta.append((Tg, S, rel, bigs))
        return meta

    meta1 = group_meta(t1, off1)
    meta2 = group_meta(t2, off2)
    SMAX = max(max(S[0] + S[1] for _, S, _, _ in meta1),
               max(max(S) for _, S, _, _ in meta2))

    with tile.TileContext(nc) as tc:
        with (
            tc.tile_pool(name="const", bufs=1) as cp,
            tc.tile_pool(name="stage", bufs=1) as stp,
            tc.tile_pool(name="sbuf", bufs=3) as sb,
            tc.tile_pool(name="gf", bufs=2) as gp,      # f32 gathered (L1)
            tc.tile_pool(name="gb", bufs=2) as gb,      # bf16 converted / L2 gathered
            tc.tile_pool(name="ob", bufs=2) as ob,      # one-hot blocks
            tc.tile_pool(name="psum", bufs=2, space="PSUM") as ps,
        ):
            nc.gpsimd.load_library(mlp)

            idx1_sb = cp.tile([P, T1 * 8], mybir.dt.int16)
            nc.sync.dma_start(out=idx1_sb[:], in_=t_idx1[:, :])
            idx2_sb = cp.tile([P, T2 * 8], mybir.dt.int16)
            nc.sync.dma_start(out=idx2_sb[:], in_=t_idx2[:, :])
            # dr/sc as [P, 1, T] so middle-dim broadcast slices need no None-axis
            dr1_sb = cp.tile([P, 1, T1], BF16)
            nc.sync.dma_start(out=dr1_sb[:, 0, :], in_=t_dr1[:, :])
            sc1_sb = cp.tile([P, 1, T1], BF16)
            nc.sync.dma_start(out=sc1_sb[:, 0, :], in_=t_sc1[:, :])
            dr2_sb = cp.tile([P, 1, T2], BF16)
            nc.sync.dma_start(out=dr2_sb[:, 0, :], in_=t_dr2[:, :])
            sc2_sb = cp.tile([P, 1, T2], BF16)
            nc.sync.dma_start(out=sc2_sb[:, 0, :], in_=t_sc2[:, :])

            disw_sb = cp.tile([P, NBLK], F32)
            nc.sync.dma_start(out=disw_sb[:, :], in_=t_disw[:, :])
            disw_bf = cp.tile([P, NBLK], BF16)
            nc.vector.tensor_copy(out=disw_bf[:], in_=disw_sb[:])

            iota_i = cp.tile([P, P], mybir.dt.int32)
            nc.gpsimd.iota(iota_i[:], pattern=[[1, P]], base=0, channel_multiplier=0)
            iota_bf = cp.tile([P, 1, P], BF16)
            nc.vector.tensor_copy(out=iota_bf[:, 0, :], in_=iota_i[:])
            # materialized iota over the dst (middle) axis: val[p, d, j] = d
            iota_big = cp.tile([P, P, SMAX], BF16)
            nc.vector.tensor_copy(
                out=iota_big[:],
                in_=iota_bf[:, 0, :][:, :, None].to_broadcast([P, P, SMAX]))

            ident_bf = cp.tile([P, P], BF16)
            make_identity(nc, ident_bf[:])
            ident_f = cp.tile([OUT_CH, OUT_CH], F32)
            make_identity(nc, ident_f[:])

            w1_f = cp.tile([IN_CH, HID_CH], F32)
            nc.sync.dma_start(out=w1_f[:], in_=t_w1[:, :])
            w1_bf = cp.tile([IN_CH, HID_CH], BF16)
            nc.vector.tensor_copy(out=w1_bf[:], in_=w1_f[:])
            w2_f = cp.tile([HID_CH, OUT_CH], F32)
            nc.sync.dma_start(out=w2_f[:], in_=t_w2[:, :])
            w2_bf = cp.tile([HID_CH, OUT_CH], BF16)
            nc.vector.tensor_copy(out=w2_bf[:], in_=w2_f[:])
            b1_sb = cp.tile([HID_CH, 1], F32)
            nc.sync.dma_start(out=b1_sb[:], in_=t_b1[:, :])
            b2_sb = cp.tile([OUT_CH, 1], F32)
            nc.sync.dma_start(out=b2_sb[:], in_=t_b2[:, :])

            # self-loop stage for L1: own-shard Dis*x in bf16 (dis_dst and one
            # dis factor live in the one-hot / dg=ident path: contribution is
            # (dis*x)[n] via ident, times dis[n] folded -> handled by dg=disw
            # ... here we pre-scale by disw so selfloop rhs can be ident*disw?
            # Keep: x1stage = Dis * x; selfloop rhs = ident scaled by disw on
            # the fly is avoided by using dg = ident_bf and folding the second
            # dis into x1stage? No: need dis^2 total; x1stage carries one dis,
            # dg carries the other (built per block below for L1 only).
            x1stage = cp.tile([P, NBLK, IN_CH], BF16)
            QS = [(0, 13), (13, 12), (25, 12), (37, 12)]
            for q0, qn in QS:
                sbx = stp.tile([P, 13, IN_CH], F32, tag="sbx")
                if q0 + qn == NBLK:
                    nc.vector.memset(sbx[:, qn - 1, :], 0.0)
                    nc.sync.dma_start(
                        out=sbx[:, :qn - 1, :],
                        in_=t_xsh[q0 * P: (q0 + qn - 1) * P, :]
                        .rearrange("(b p) c -> p b c", p=P))
                    nc.sync.dma_start(out=sbx[:NTAIL, qn - 1, :],
                                      in_=t_xsh[(q0 + qn - 1) * P:, :])
                else:
                    nc.sync.dma_start(
                        out=sbx[:, :qn, :],
                        in_=t_xsh[q0 * P: (q0 + qn) * P, :]
                        .rearrange("(b p) c -> p b c", p=P))
                nc.vector.tensor_tensor(
                    out=x1stage[:, q0:q0 + qn, :], in0=sbx[:, :qn, :],
                    in1=disw_sb[:, q0:q0 + qn][:, :, None]
                    .to_broadcast([P, qn, IN_CH]),
                    op=mybir.AluOpType.mult)

            x2st0 = cp.tile([P, CLEN[0], HID_CH], BF16)
            x2st1 = cp.tile([P, CLEN[1], HID_CH], BF16)
            x2st = [x2st0, x2st1]

            def x2stage_slice(b):
                c = 0 if b < CSTART[1] else 1
                return x2st[c][:, b - CSTART[c], :]

            acc2 = cp.tile([HID_CH, NBLK, P], BF16)

            gq = [0]
            tabv1 = t_x.ap()

            def gather(out_ap, in_ap, idx_sb, t0, n):
                o0 = 0
                while o0 < n:
                    m = min(GCAP, n - o0)
                    nc.gpsimd.dma_gather(
                        out_ap=out_ap[:, o0:o0 + m, :], in_ap=in_ap,
                        idxs_ap=idx_sb[:, 8 * (t0 + o0): 8 * (t0 + o0 + m)],
                        num_idxs=m * P, num_idxs_reg=m * P, elem_size=IN_CH,
                        queue_num=gq[0] % 4,
                    )
                    gq[0] += 1
                    o0 += m

            def onehot(S, dr_sb, sc_sb, Tg):
                """[P(edge), P(dst), S(tile)] one-hot scaled by per-edge sc."""
                o3 = ob.tile([P, P, S], BF16, tag="o3")
                nc.vector.tensor_tensor(
                    out=o3[:], in0=iota_big[:, :, :S],
                    in1=dr_sb[:, :, Tg:Tg + S].to_broadcast([P, P, S]),
                    op=mybir.AluOpType.is_equal,
                )
                nc.vector.tensor_tensor(
                    out=o3[:], in0=o3[:],
                    in1=sc_sb[:, :, Tg:Tg + S].to_broadcast([P, P, S]),
                    op=mybir.AluOpType.mult,
                )
                return o3

            def dg_tile(b, dsc):
                dg = sb.tile([P, P], BF16, tag="dg")
                nc.vector.tensor_tensor(
                    out=dg[:], in0=ident_bf[:],
                    in1=dsc[:, b: b + 1].to_broadcast([P, P]),
                    op=mybir.AluOpType.mult,
                )
                return dg

            def epi1(b, ups):
                h1t = sb.tile([HID_CH, P], BF16, tag="h1t")
                nc.scalar.activation(out=h1t[:], in_=ups[:],
                                     func=mybir.ActivationFunctionType.Relu,
                                     bias=b1_sb[:, :1])
                trp = ps.tile([P, HID_CH], BF16, tag="trp")
                nc.tensor.transpose(out=trp[:], in_=h1t[:], identity=ident_bf[:])
                nc.vector.tensor_tensor(
                    out=x2stage_slice(b), in0=trp[:],
                    in1=disw_sb[:, b: b + 1].to_broadcast([P, HID_CH]),
                    op=mybir.AluOpType.mult,
                )

            def epi2(b, nb, ups):
                h2t = sb.tile([OUT_CH, P], F32, tag="h2t")
                nc.scalar.activation(out=h2t[:], in_=ups[:],
                                     func=mybir.ActivationFunctionType.Relu,
                                     bias=b2_sb[:, :1])
                trp2 = ps.tile([P, OUT_CH], F32, tag="trp2")
                nc.tensor.transpose(out=trp2[:], in_=h2t[:], identity=ident_f[:])
                outt = sb.tile([P, OUT_CH], F32, tag="outt")
                nc.vector.tensor_copy(out=outt[:], in_=trp2[:])
                nc.sync.dma_start(out=t_out[b * P: b * P + nb, :], in_=outt[:nb, :])

            # ================= Layer 1 =================
            def l1_group(g):
                Tg, S, rel, bigs = meta1[g]
                Sg = S[0] + S[1]
                g1 = gp.tile([P, Sg, IN_CH], F32, tag="g1")
                if S[0]:
                    gather(g1[:, :S[0], :], tabv1[0:VLO, :], idx1_sb, Tg, S[0])
                if S[1]:
                    gather(g1[:, S[0]:, :], tabv1[VLO:N_NODES, :], idx1_sb,
                           Tg + S[0], S[1])
                g1b = gb.tile([P, Sg, IN_CH], BF16, tag="gb")
                nc.scalar.copy(
                    out=g1b[:].rearrange("p s c -> p (s c)"),
                    in_=g1[:].rearrange("p s c -> p (s c)"))
                o3 = onehot(Sg, dr1_sb, sc1_sb, Tg)
                for big in bigs:
                    b = g * GS + big
                    tps = ps.tile([HID_CH, P], F32, tag="tps")
                    js = []
                    for h in range(2):
                        p0, n = rel[(h, big)]
                        js += list(range(p0, p0 + n))
                    for ji, j in enumerate(js):
                        nc.tensor.matmul(out=tps[:], lhsT=g1b[:, j, :],
                                         rhs=o3[:, :, j],
                                         start=(ji == 0), stop=False)
                    dg = dg_tile(b, disw_bf)
                    nc.tensor.matmul(out=tps[:], lhsT=x1stage[:, b, :], rhs=dg[:],
                                     start=(len(js) == 0), stop=True)
                    t_sb = sb.tile([HID_CH, P], BF16, tag="tsb")
                    nc.scalar.copy(out=t_sb[:], in_=tps[:])
                    ups = ps.tile([HID_CH, P], F32, tag="ups")
                    nc.tensor.matmul(out=ups[:], lhsT=w1_bf[:], rhs=t_sb[:],
                                     start=True, stop=True)
                    epi1(b, ups)

            for g in range(AG0_AFTER_GROUP):
                l1_group(g)
            # chunk-0 x2' ready (blocks 0..24 done in groups 0..12): exchange it
            # while the rest of layer 1 runs
            nc.sync.dma_start(
                out=x2s[0][:, :], in_=x2st[0][:].rearrange("p b c -> p (b c)"))
            # emit collectives on the SP / ACT sequencers: on the Pool engine
            # they would stall all later gather desc-gen for their duration
            nc.gpsimd.collective_compute(
                "AllGather", mybir.AluOpType.bypass, replica_groups=rg,
                ins=[x2s[0].ap().opt()], outs=[x2f[0].ap().opt()],
            )
            for g in range(AG0_AFTER_GROUP, NG):
                l1_group(g)
            nc.sync.dma_start(
                out=x2s[1][:, :], in_=x2st[1][:].rearrange("p b c -> p (b c)"))
            nc.gpsimd.collective_compute(
                "AllGather", mybir.AluOpType.bypass, replica_groups=rg,
                ins=[x2s[1].ap().opt()], outs=[x2f[1].ap().opt()],
            )

            # ================= Layer 2 (two passes over chunk tables) ========
            tabv2 = [x2f[c].ap().rearrange("q (r c) -> (q r) c", c=HID_CH)
                     for c in range(2)]

            for cpass in range(2):
                for g in range(NG):
                    Tg, S, rel, bigs = meta2[g]
                    Sc = S[cpass]
                    base = Tg + (S[0] if cpass == 1 else 0)
                    if Sc:
                        g2 = gb.tile([P, Sc, HID_CH], BF16, tag="gb")
                        gather(g2[:, :, :], tabv2[cpass], idx2_sb, base, Sc)
                        o3 = onehot(Sc, dr2_sb, sc2_sb, base)
                    for big in bigs:
                        b = g * GS + big
                        p0, n = rel[(cpass, big)]
                        p0 -= (S[0] if cpass == 1 else 0)
                        if cpass == 0:
                            if n == 0:
                                nc.vector.memset(acc2[:, b, :], 0.0)
                                continue
                            tps = ps.tile([HID_CH, P], F32, tag="tps")
                            for ji in range(n):
                                nc.tensor.matmul(out=tps[:], lhsT=g2[:, p0 + ji, :],
                                                 rhs=o3[:, :, p0 + ji],
                                                 start=(ji == 0), stop=(ji == n - 1))
                            nc.vector.tensor_copy(out=acc2[:, b, :], in_=tps[:])
                        else:
                            tps = ps.tile([HID_CH, P], F32, tag="tps")
                            for ji in range(n):
                                nc.tensor.matmul(out=tps[:], lhsT=g2[:, p0 + ji, :],
                                                 rhs=o3[:, :, p0 + ji],
                                                 start=(ji == 0), stop=False)
                            dg = dg_tile(b, disw_bf)
                            nc.tensor.matmul(out=tps[:], lhsT=x2stage_slice(b),
                                             rhs=dg[:], start=(n == 0), stop=True)
                            t_sb = sb.tile([HID_CH, P], BF16, tag="tsb")
                            nc.vector.tensor_tensor(
                                out=t_sb[:], in0=tps[:], in1=acc2[:, b, :],
                                op=mybir.AluOpType.add)
                            ups = ps.tile([OUT_CH, P], F32, tag="ups")
                            nc.tensor.matmul(out=ups[:], lhsT=w2_bf[:], rhs=t_sb[:],
                                             start=True, stop=True)
                            nb = P if b < NFULL else NTAIL
                            epi2(b, nb, ups)

    nc.compile()
    return nc


def kernel(x, edge_index, W1, b1, W2, b2, _trace=False):
    global LAST_RESULT
    x = np.asarray(x, dtype=np.float32)
    edge_index = np.asarray(edge_index, dtype=np.int32)

    prep = _host_prep(edge_index)
    nc = _build(prep)
    in_maps = _make_in_maps(x, W1, b1, W2, b2, prep)

    res = run_bass_kernel_spmd(nc, in_maps, core_ids=list(range(N_CORES)),
                               trace=_trace)
    LAST_RESULT = res
    out = np.concatenate([res.results[k]["out"] for k in range(N_CORES)], axis=0)
    return out.astype(np.float32)


# revision 27
# speedup vs baseline: 1.0638x; 1.0638x over previous
"""2-layer GCN (GCNConv 128->128->64, N=50000, E=800000) on 8 TRN2 NeuronCores.

v4 strategy (dst-sharded, aggregate-first, gather-direct):
  out = relu(A_hat @ (relu(A_hat @ x @ W1 + b1)) @ W2 + b2),  A_hat = D^-1/2 (A+I) D^-1/2
  - Layer 1 gathers source features DIRECTLY from the replicated f32 input x
    (512B rows, no table build, no AllGather #1). ScalarE converts f32->bf16.
  - Per-edge norm folded into the one-hot scatter matrices (VectorE, 2x mode:
    one-hot laid out [edge, dst, tile] so every operand has a packed last dim,
    iota materialized as a const [P, P, SMAX] tile).
  - Layer 2: x2' = Dis*relu(h1) computed dst-sharded, exchanged via TWO chunked
    AllGathers (src-block ranges 0:25 / 25:49); AllGather #0 is emitted into the
    Pool stream mid-layer-1 so it overlaps the layer-1 tail; layer 2 runs two
    passes (one per chunk table) accumulating into a bf16 SBUF accumulator.
  - Edges sorted by dst, sharded across 8 cores by dst range (6250/core), dst
    blocks of 128; scatter-add via PSUM-accumulated TensorE matmuls.
  - Gathers in 8-tile (1024-idx) dma_gather calls (HW packet limit: 64
    descriptors/engine), 4 SWDGE queues round-robin.
Host-side work is index-only prep (sort/degree-histogram/plan) + output concat.
"""

import numpy as np
import ml_dtypes

import concourse.bass as bass
import concourse.bacc as bacc
import concourse.mybir as mybir
import concourse.tile as tile
from concourse.bass_utils import run_bass_kernel_spmd
from concourse.library_config import mlp
from concourse.masks import make_identity

P = 128
N_NODES = 50000
N_EDGES = 800000
IN_CH = 128
HID_CH = 128
OUT_CH = 64
N_CORES = 8
NSH = N_NODES // N_CORES          # 6250 nodes per core
NBLK = (NSH + P - 1) // P         # 49 dst blocks per core (48 full + 106)
NFULL = NSH // P                  # 48
NTAIL = NSH - NFULL * P           # 106
VLO = 32768                       # layer-1 table halves (int16 index range)
GS = 2                            # dst blocks per gather group
NG = (NBLK + GS - 1) // GS        # 25 groups (last has 1 block)
GCAP = 8                          # max tiles (128 idxs each) per dma_gather call
CLEN = [25, 24]                   # layer-2 src-block chunks (blocks 0:25, 25:49)
CSTART = [0, 25]
AG0_AFTER_GROUP = 13              # emit chunk-0 AllGather after this many groups

BF16 = mybir.dt.bfloat16
F32 = mybir.dt.float32

LAST_RESULT = None  # for test harness: BassKernelResults of last run


def _plan(cnt):
    t = np.ceil(cnt.max(axis=0) / P).astype(np.int64)
    off = np.concatenate([[0], np.cumsum(t)])[:-1]
    return t, off, int(t.sum())


def _host_prep(edge_index):
    """Index-only preprocessing. Returns per-core upload arrays + tile plans."""
    src = edge_index[0].astype(np.int64)
    dst = edge_index[1].astype(np.int64)

    deg = np.bincount(dst, minlength=N_NODES) + 1
    dis = (1.0 / np.sqrt(deg.astype(np.float64))).astype(np.float64)

    core = dst // NSH
    i_dst = dst - core * NSH
    blk = i_dst // P
    grp = blk // GS
    big = blk - grp * GS

    # ---- layer 1: table is x itself, halves at VLO ----
    half = (src >= VLO).astype(np.int64)
    row1 = np.where(half == 1, src - VLO, src)
    kk1 = ((core * NG + grp) * 2 + half) * GS + big      # [E]
    NSLOT = NG * 2 * GS                                   # 100 per core
    cnt1 = np.zeros((N_CORES, NSLOT), np.int64)
    np.add.at(cnt1, (core, kk1 - core * NSLOT), 1)
    t1, off1, T1 = _plan(cnt1)

    # ---- layer 2: chunked tables; row = (ksrc*128+psrc)*len_c + (bsrc-cstart) ----
    ksrc = src // NSH
    isrc = src - ksrc * NSH
    bsrc = isrc // P
    psrc = isrc - bsrc * P
    ch = (bsrc >= CLEN[0]).astype(np.int64)
    lenc = np.where(ch == 1, CLEN[1], CLEN[0])
    row2 = (ksrc * P + psrc) * lenc + (bsrc - ch * CSTART[1])
    kk2 = ((core * NG + grp) * 2 + ch) * GS + big
    cnt2 = np.zeros((N_CORES, NSLOT), np.int64)
    np.add.at(cnt2, (core, kk2 - core * NSLOT), 1)
    t2, off2, T2 = _plan(cnt2)

    sc_l1 = (dis[src] * dis[dst]).astype(np.float32)   # table1 = raw x
    sc_l2 = dis[dst].astype(np.float32)                # table2 = Dis*relu(h1)
    drel = (dst - (core * NSH + blk * P)).astype(np.float32)

    def build(kkey, cnt, tiles, offs, T, rows, sc):
        order = np.argsort(kkey, kind="stable")
        ks = kkey[order]
        group_start = np.concatenate([[0], np.cumsum(cnt.reshape(-1))])[:-1]
        pos = np.arange(len(ks)) - group_start[ks]
        slot_in_core = offs[ks % NSLOT] * P + pos
        ecore = ks // NSLOT
        EPC = T * P
        idx_rows = np.zeros((N_CORES, EPC), np.int64)
        dr = np.full((N_CORES, EPC), -1.0, np.float32)
        scl = np.zeros((N_CORES, EPC), np.float32)
        idx_rows[ecore, slot_in_core] = rows[order]
        dr[ecore, slot_in_core] = drel[order]
        scl[ecore, slot_in_core] = sc[order]
        idxw = np.zeros((N_CORES, 16, T * 8), np.int16)
        ii = np.arange(EPC)
        for k in range(N_CORES):
            w = np.zeros((16, T * 8), np.int16)
            w[ii % 16, ii // 16] = idx_rows[k]
            idxw[k] = w
        idxw = np.tile(idxw, (1, 8, 1))
        drw = dr.reshape(N_CORES, T, P).transpose(0, 2, 1)
        scw = scl.reshape(N_CORES, T, P).transpose(0, 2, 1)
        return idxw.astype(np.int16), drw, scw

    idxw1, dr1, sc1 = build(kk1, cnt1, t1, off1, T1, row1, sc_l1)
    idxw2, dr2, sc2 = build(kk2, cnt2, t2, off2, T2, row2, sc_l2)

    disw = np.zeros((N_CORES, P, NBLK), np.float32)
    nodes = np.arange(NBLK * P)
    valid = nodes < NSH
    disf = dis.astype(np.float32)
    for k in range(N_CORES):
        v = np.zeros(NBLK * P, np.float32)
        v[valid] = disf[k * NSH + nodes[valid]]
        disw[k] = v.reshape(NBLK, P).T

    return {
        "t1": t1, "off1": off1, "T1": T1,
        "t2": t2, "off2": off2, "T2": T2,
        "idxw1": idxw1, "dr1": dr1.astype(ml_dtypes.bfloat16),
        "sc1": sc1.astype(ml_dtypes.bfloat16),
        "idxw2": idxw2, "dr2": dr2.astype(ml_dtypes.bfloat16),
        "sc2": sc2.astype(ml_dtypes.bfloat16),
        "disw": disw,
    }


def _make_in_maps(x, W1, b1, W2, b2, prep):
    in_maps = []
    xf = np.ascontiguousarray(x, dtype=np.float32)
    for k in range(N_CORES):
        in_maps.append({
            "x": xf,
            "xsh": np.ascontiguousarray(xf[k * NSH:(k + 1) * NSH]),
            "w1": np.ascontiguousarray(W1, dtype=np.float32),
            "b1": np.ascontiguousarray(np.asarray(b1, np.float32).reshape(HID_CH, 1)),
            "w2": np.ascontiguousarray(W2, dtype=np.float32),
            "b2": np.ascontiguousarray(np.asarray(b2, np.float32).reshape(OUT_CH, 1)),
            "disw": np.ascontiguousarray(prep["disw"][k]),
            "idxw1": np.ascontiguousarray(prep["idxw1"][k]),
            "dr1": np.ascontiguousarray(prep["dr1"][k]),
            "sc1": np.ascontiguousarray(prep["sc1"][k]),
            "idxw2": np.ascontiguousarray(prep["idxw2"][k]),
            "dr2": np.ascontiguousarray(prep["dr2"][k]),
            "sc2": np.ascontiguousarray(prep["sc2"][k]),
        })
    return in_maps


def _build(prep):
    t1, off1, T1 = prep["t1"], prep["off1"], prep["T1"]
    t2, off2, T2 = prep["t2"], prep["off2"], prep["T2"]

    nc = bacc.Bacc("TRN2", target_bir_lowering=False, num_devices=N_CORES,
                   num_swdge_queues=4)

    t_x = nc.dram_tensor("x", [N_NODES, IN_CH], F32, kind="ExternalInput")
    t_xsh = nc.dram_tensor("xsh", [NSH, IN_CH], F32, kind="ExternalInput")
    t_w1 = nc.dram_tensor("w1", [IN_CH, HID_CH], F32, kind="ExternalInput")
    t_b1 = nc.dram_tensor("b1", [HID_CH, 1], F32, kind="ExternalInput")
    t_w2 = nc.dram_tensor("w2", [HID_CH, OUT_CH], F32, kind="ExternalInput")
    t_b2 = nc.dram_tensor("b2", [OUT_CH, 1], F32, kind="ExternalInput")
    t_disw = nc.dram_tensor("disw", [P, NBLK], F32, kind="ExternalInput")
    t_idx1 = nc.dram_tensor("idxw1", [P, T1 * 8], mybir.dt.int16, kind="ExternalInput")
    t_dr1 = nc.dram_tensor("dr1", [P, T1], BF16, kind="ExternalInput")
    t_sc1 = nc.dram_tensor("sc1", [P, T1], BF16, kind="ExternalInput")
    t_idx2 = nc.dram_tensor("idxw2", [P, T2 * 8], mybir.dt.int16, kind="ExternalInput")
    t_dr2 = nc.dram_tensor("dr2", [P, T2], BF16, kind="ExternalInput")
    t_sc2 = nc.dram_tensor("sc2", [P, T2], BF16, kind="ExternalInput")
    t_out = nc.dram_tensor("out", [NSH, OUT_CH], F32, kind="ExternalOutput")

    x2s = [nc.dram_tensor(f"x2s_{c}", [P, CLEN[c] * HID_CH], BF16) for c in range(2)]
    x2f = [nc.dram_tensor(f"x2f_{c}", [N_CORES * P, CLEN[c] * HID_CH], BF16)
           for c in range(2)]

    rg = [list(range(N_CORES))]

    def slot(g, h, big):
        return (g * 2 + h) * GS + big

    def group_meta(tiles, offs):
        meta = []
        for g in range(NG):
            bigs = list(range(min(GS, NBLK - g * GS)))
            Tg = int(offs[slot(g, 0, 0)])
            S = [0, 0]
            rel = {}
            pos = 0
            for h in range(2):
                for big in bigs:
                    n = int(tiles[slot(g, h, big)])
                    rel[(h, big)] = (pos, n)
                    S[h] += n
                    pos += n
            meta.append((Tg, S, rel, bigs))
        return meta

    meta1 = group_meta(t1, off1)
    meta2 = group_meta(t2, off2)
    SMAX = max(max(S[0] + S[1] for _, S, _, _ in meta1),
               max(max(S) for _, S, _, _ in meta2))

    with tile.TileContext(nc) as tc:
        with (
            tc.tile_pool(name="const", bufs=1) as cp,
            tc.tile_pool(name="stage", bufs=1) as stp,
            tc.tile_pool(name="sbuf", bufs=3) as sb,
            tc.tile_pool(name="gf", bufs=2) as gp,      # f32 gathered (L1)
            tc.tile_pool(name="gb", bufs=2) as gb,      # bf16 converted / L2 gathered
            tc.tile_pool(name="ob", bufs=2) as ob,      # one-hot blocks
            tc.tile_pool(name="psum", bufs=2, space="PSUM") as ps,
        ):
            nc.gpsimd.load_library(mlp)

            idx1_sb = cp.tile([P, T1 * 8], mybir.dt.int16)
            nc.sync.dma_start(out=idx1_sb[:], in_=t_idx1[:, :])
            idx2_sb = cp.tile([P, T2 * 8], mybir.dt.int16)
            nc.sync.dma_start(out=idx2_sb[:], in_=t_idx2[:, :])
            # dr/sc as [P, 1, T] so middle-dim broadcast slices need no None-axis
            dr1_sb = cp.tile([P, 1, T1], BF16)
            nc.sync.dma_start(out=dr1_sb[:, 0, :], in_=t_dr1[:, :])
            sc1_sb = cp.tile([P, 1, T1], BF16)
            nc.sync.dma_start(out=sc1_sb[:, 0, :], in_=t_sc1[:, :])
            dr2_sb = cp.tile([P, 1, T2], BF16)
            nc.sync.dma_start(out=dr2_sb[:, 0, :], in_=t_dr2[:, :])
            sc2_sb = cp.tile([P, 1, T2], BF16)
            nc.sync.dma_start(out=sc2_sb[:, 0, :], in_=t_sc2[:, :])

            disw_sb = cp.tile([P, NBLK], F32)
            nc.sync.dma_start(out=disw_sb[:, :], in_=t_disw[:, :])
            disw_bf = cp.tile([P, NBLK], BF16)
            nc.vector.tensor_copy(out=disw_bf[:], in_=disw_sb[:])

            iota_i = cp.tile([P, P], mybir.dt.int32)
            nc.gpsimd.iota(iota_i[:], pattern=[[1, P]], base=0, channel_multiplier=0)
            iota_bf = cp.tile([P, 1, P], BF16)
            nc.vector.tensor_copy(out=iota_bf[:, 0, :], in_=iota_i[:])
            # materialized iota over the dst (middle) axis: val[p, d, j] = d
            iota_big = cp.tile([P, P, SMAX], BF16)
            nc.vector.tensor_copy(
                out=iota_big[:],
                in_=iota_bf[:, 0, :][:, :, None].to_broadcast([P, P, SMAX]))

            ident_bf = cp.tile([P, P], BF16)
            make_identity(nc, ident_bf[:])
            ident_f = cp.tile([OUT_CH, OUT_CH], F32)
            make_identity(nc, ident_f[:])

            w1_f = cp.tile([IN_CH, HID_CH], F32)
            nc.sync.dma_start(out=w1_f[:], in_=t_w1[:, :])
            w1_bf = cp.tile([IN_CH, HID_CH], BF16)
            nc.vector.tensor_copy(out=w1_bf[:], in_=w1_f[:])
            w2_f = cp.tile([HID_CH, OUT_CH], F32)
            nc.sync.dma_start(out=w2_f[:], in_=t_w2[:, :])
            w2_bf = cp.tile([HID_CH, OUT_CH], BF16)
            nc.vector.tensor_copy(out=w2_bf[:], in_=w2_f[:])
            b1_sb = cp.tile([HID_CH, 1], F32)
            nc.sync.dma_start(out=b1_sb[:], in_=t_b1[:, :])
            b2_sb = cp.tile([OUT_CH, 1], F32)
            nc.sync.dma_start(out=b2_sb[:], in_=t_b2[:, :])

            # L1 self-loop stage: own-shard Dis*x in bf16. Each self loop needs
            # dis^2 total: one factor here, the other via dg = ident*disw.
            x1stage = cp.tile([P, NBLK, IN_CH], BF16)
            QS = [(0, 13), (13, 12), (25, 12), (37, 12)]
            for q0, qn in QS:
                sbx = stp.tile([P, 13, IN_CH], F32, tag="sbx")
                if q0 + qn == NBLK:
                    nc.vector.memset(sbx[:, qn - 1, :], 0.0)
                    nc.sync.dma_start(
                        out=sbx[:, :qn - 1, :],
                        in_=t_xsh[q0 * P: (q0 + qn - 1) * P, :]
                        .rearrange("(b p) c -> p b c", p=P))
                    nc.sync.dma_start(out=sbx[:NTAIL, qn - 1, :],
                                      in_=t_xsh[(q0 + qn - 1) * P:, :])
                else:
                    nc.sync.dma_start(
                        out=sbx[:, :qn, :],
                        in_=t_xsh[q0 * P: (q0 + qn) * P, :]
                        .rearrange("(b p) c -> p b c", p=P))
                nc.vector.tensor_tensor(
                    out=x1stage[:, q0:q0 + qn, :], in0=sbx[:, :qn, :],
                    in1=disw_sb[:, q0:q0 + qn][:, :, None]
                    .to_broadcast([P, qn, IN_CH]),
                    op=mybir.AluOpType.mult)

            x2st0 = cp.tile([P, CLEN[0], HID_CH], BF16)
            x2st1 = cp.tile([P, CLEN[1], HID_CH], BF16)
            x2st = [x2st0, x2st1]

            def x2stage_slice(b):
                c = 0 if b < CSTART[1] else 1
                return x2st[c][:, b - CSTART[c], :]

            acc2 = cp.tile([HID_CH, NBLK, P], BF16)

            gq = [0]
            tabv1 = t_x.ap()

            def gather(out_ap, in_ap, idx_sb, t0, n):
                o0 = 0
                while o0 < n:
                    m = min(GCAP, n - o0)
                    nc.gpsimd.dma_gather(
                        out_ap=out_ap[:, o0:o0 + m, :], in_ap=in_ap,
                        idxs_ap=idx_sb[:, 8 * (t0 + o0): 8 * (t0 + o0 + m)],
                        num_idxs=m * P, num_idxs_reg=m * P, elem_size=IN_CH,
                        queue_num=gq[0] % 4,
                    )
                    gq[0] += 1
                    o0 += m

            def onehot(S, dr_sb, sc_sb, Tg):
                """[P(edge), P(dst), S(tile)] one-hot scaled by per-edge sc."""
                o3 = ob.tile([P, P, S], BF16, tag="o3")
                nc.vector.tensor_tensor(
                    out=o3[:], in0=iota_big[:, :, :S],
                    in1=dr_sb[:, :, Tg:Tg + S].to_broadcast([P, P, S]),
                    op=mybir.AluOpType.is_equal,
                )
                nc.vector.tensor_tensor(
                    out=o3[:], in0=o3[:],
                    in1=sc_sb[:, :, Tg:Tg + S].to_broadcast([P, P, S]),
                    op=mybir.AluOpType.mult,
                )
                return o3

            def dg_tile(b, dsc):
                dg = sb.tile([P, P], BF16, tag="dg")
                nc.vector.tensor_tensor(
                    out=dg[:], in0=ident_bf[:],
                    in1=dsc[:, b: b + 1].to_broadcast([P, P]),
                    op=mybir.AluOpType.mult,
                )
                return dg

            def epi1(b, ups):
                h1t = sb.tile([HID_CH, P], BF16, tag="h1t")
                nc.scalar.activation(out=h1t[:], in_=ups[:],
                                     func=mybir.ActivationFunctionType.Relu,
                                     bias=b1_sb[:, :1])
                trp = ps.tile([P, HID_CH], BF16, tag="trp")
                nc.tensor.transpose(out=trp[:], in_=h1t[:], identity=ident_bf[:])
                nc.vector.tensor_tensor(
                    out=x2stage_slice(b), in0=trp[:],
                    in1=disw_sb[:, b: b + 1].to_broadcast([P, HID_CH]),
                    op=mybir.AluOpType.mult,
                )

            def epi2(b, nb, ups):
                h2t = sb.tile([OUT_CH, P], F32, tag="h2t")
                nc.scalar.activation(out=h2t[:], in_=ups[:],
                                     func=mybir.ActivationFunctionType.Relu,
                                     bias=b2_sb[:, :1])
                trp2 = ps.tile([P, OUT_CH], F32, tag="trp2")
                nc.tensor.transpose(out=trp2[:], in_=h2t[:], identity=ident_f[:])
                outt = sb.tile([P, OUT_CH], F32, tag="outt")
                nc.vector.tensor_copy(out=outt[:], in_=trp2[:])
                nc.sync.dma_start(out=t_out[b * P: b * P + nb, :], in_=outt[:nb, :])

            # ================= Layer 1 =================
            def l1_group(g):
                Tg, S, rel, bigs = meta1[g]
                Sg = S[0] + S[1]
                g1 = gp.tile([P, Sg, IN_CH], F32, tag="g1")
                if S[0]:
                    gather(g1[:, :S[0], :], tabv1[0:VLO, :], idx1_sb, Tg, S[0])
                if S[1]:
                    gather(g1[:, S[0]:, :], tabv1[VLO:N_NODES, :], idx1_sb,
                           Tg + S[0], S[1])
                g1b = gb.tile([P, Sg, IN_CH], BF16, tag="gb")
                nc.scalar.copy(
                    out=g1b[:].rearrange("p s c -> p (s c)"),
                    in_=g1[:].rearrange("p s c -> p (s c)"))
                o3 = onehot(Sg, dr1_sb, sc1_sb, Tg)
                for big in bigs:
                    b = g * GS + big
                    tps = ps.tile([HID_CH, P], F32, tag="tps")
                    js = []
                    for h in range(2):
                        p0, n = rel[(h, big)]
                        js += list(range(p0, p0 + n))
                    for ji, j in enumerate(js):
                        nc.tensor.matmul(out=tps[:], lhsT=g1b[:, j, :],
                                         rhs=o3[:, :, j],
                                         start=(ji == 0), stop=False)
                    dg = dg_tile(b, disw_bf)
                    nc.tensor.matmul(out=tps[:], lhsT=x1stage[:, b, :], rhs=dg[:],
                                     start=(len(js) == 0), stop=True)
                    t_sb = sb.tile([HID_CH, P], BF16, tag="tsb")
                    nc.scalar.copy(out=t_sb[:], in_=tps[:])
                    ups = ps.tile([HID_CH, P], F32, tag="ups")
                    nc.tensor.matmul(out=ups[:], lhsT=w1_bf[:], rhs=t_sb[:],
                                     start=True, stop=True)
                    epi1(b, ups)

            for g in range(AG0_AFTER_GROUP):
                l1_group(g)
            # chunk-0 x2' ready (blocks 0..24 done in groups 0..12): exchange it
            # while the rest of layer 1 runs
            nc.sync.dma_start(
                out=x2s[0][:, :], in_=x2st[0][:].rearrange("p b c -> p (b c)"))
            # emit collectives on the SP / ACT sequencers: on the Pool engine
            # they would stall all later gather desc-gen for their duration
            nc.gpsimd.collective_compute(
                "AllGather", mybir.AluOpType.bypass, replica_groups=rg,
                ins=[x2s[0].ap().opt()], outs=[x2f[0].ap().opt()],
            )
            for g in range(AG0_AFTER_GROUP, NG):
                l1_group(g)
            nc.sync.dma_start(
                out=x2s[1][:, :], in_=x2st[1][:].rearrange("p b c -> p (b c)"))
            nc.gpsimd.collective_compute(
                "AllGather", mybir.AluOpType.bypass, replica_groups=rg,
                ins=[x2s[1].ap().opt()], outs=[x2f[1].ap().opt()],
            )

            # ================= Layer 2 (two passes over chunk tables) ========
            tabv2 = [x2f[c].ap().rearrange("q (r c) -> (q r) c", c=HID_CH)
                     for c in range(2)]

            for cpass in range(2):
                for g in range(NG):
                    Tg, S, rel, bigs = meta2[g]
                    Sc = S[cpass]
                    base = Tg + (S[0] if cpass == 1 else 0)
                    if Sc:
                        g2 = gb.tile([P, Sc, HID_CH], BF16, tag="gb")
                        gather(g2[:, :, :], tabv2[cpass], idx2_sb, base, Sc)
                        o3 = onehot(Sc, dr2_sb, sc2_sb, base)
                    for big in bigs:
                        b = g * GS + big
                        p0, n = rel[(cpass, big)]
                        p0 -= (S[0] if cpass == 1 else 0)
                        if cpass == 0:
                            if n == 0:
                                nc.vector.memset(acc2[:, b, :], 0.0)
                                continue
                            tps = ps.tile([HID_CH, P], F32, tag="tps")
                            for ji in range(n):
                                nc.tensor.matmul(out=tps[:], lhsT=g2[:, p0 + ji, :],
                                                 rhs=o3[:, :, p0 + ji],
                                                 start=(ji == 0), stop=(ji == n - 1))
                            nc.vector.tensor_copy(out=acc2[:, b, :], in_=tps[:])
                        else:
                            tps = ps.tile([HID_CH, P], F32, tag="tps")
                            for ji in range(n):
                                nc.tensor.matmul(out=tps[:], lhsT=g2[:, p0 + ji, :],
                                                 rhs=o3[:, :, p0 + ji],
                                                 start=(ji == 0), stop=False)
                            dg = dg_tile(b, disw_bf)
                            nc.tensor.matmul(out=tps[:], lhsT=x2stage_slice(b),
                                             rhs=dg[:], start=(n == 0), stop=True)
                            t_sb = sb.tile([HID_CH, P], BF16, tag="tsb")
                            nc.vector.tensor_tensor(
                                out=t_sb[:], in0=tps[:], in1=acc2[:, b, :],
                                op=mybir.AluOpType.add)
                            ups = ps.tile([OUT_CH, P], F32, tag="ups")
                            nc.tensor.matmul(out=ups[:], lhsT=w2_bf[:], rhs=t_sb[:],
                                             start=True, stop=True)
                            nb = P if b < NFULL else NTAIL
                            epi2(b, nb, ups)

    nc.compile()
    return nc


def kernel(x, edge_index, W1, b1, W2, b2, _trace=False):
    global LAST_RESULT
    x = np.asarray(x, dtype=np.float32)
    edge_index = np.asarray(edge_index, dtype=np.int32)

    prep = _host_prep(edge_index)
    nc = _build(prep)
    in_maps = _make_in_maps(x, W1, b1, W2, b2, prep)

    res = run_bass_kernel_spmd(nc, in_maps, core_ids=list(range(N_CORES)),
                               trace=_trace)
    LAST_RESULT = res
    out = np.concatenate([res.results[k]["out"] for k in range(N_CORES)], axis=0)
    return out.astype(np.float32)
